# revision 4
# baseline (speedup 1.0000x reference)
"""Trainium2 Bass kernel: 8-expert top-2 MoE MLP (SwiGLU), expert-parallel on 8 cores.

Strategy (per sharding hint, expert-parallel):
  - Host: router matmul + top-2 + softmax weights (67 MFLOP — negligible),
    gather each expert's tokens into a zero-padded capacity-C buffer, staged
    TRANSPOSED ([D, C], bf16) so the device kernel needs no transposes at all.
  - Device (per core = one expert): fused SwiGLU FFN as two chained GEMMs with
    features on partitions and tokens on the moving free dim:
      H'^T[2M, C] = (W13 stationary).T-free x X^T moving  (contract D)
      H^T = silu(gate) * up                               (ACT + DVE)
      O^T[D, C]  = (W2 stationary) x H^T moving           (contract M)
  - Host: weighted scatter-add of the 8 per-expert outputs back to token order.

Weights live in SBUF for the whole kernel (12 MB bf16/core). All matmuls are
bf16 with fp32 PSUM accumulation (rel err ~5e-3 vs fp32 reference).
"""

from contextlib import ExitStack

import ml_dtypes
import numpy as np

import concourse.bass as bass  # noqa: F401  (AP helpers)
import concourse.tile as tile
from concourse import bacc, mybir
from concourse.bass_utils import run_bass_kernel_spmd

# nn_MoEMLP_82617990905863 (hardcoded per contract)
B, S, D = 4, 2048, 1024
T = B * S               # 8192 tokens
E = 8                   # experts == cores
TOPK = 2
M = 2048                # MOE_DIM (w13 = [D, 2M], w2 = [M, D])
TB = 512                # token block = max moving free dim
KD = D // 128           # 8 contraction tiles for X @ W13
KH = M // 128           # 16 contraction tiles for H @ W2

_NC_CACHE: dict[int, object] = {}
last_results = None     # BassKernelResults of the most recent run (for test.py)


def _build(C: int, use_silu: bool = True):
    """Build + compile the SPMD per-core graph for capacity C (multiple of 128).

    use_silu=False decomposes silu as g*sigmoid(g) (CoreSim lacks the Silu LUT).
    """
    dt = mybir.dt
    nc = bacc.Bacc(
        "TRN2", target_bir_lowering=False, debug=False, enable_asserts=False
    )
    xt = nc.dram_tensor("xt", [D, C], dt.bfloat16, kind="ExternalInput").ap()
    w13 = nc.dram_tensor("w13", [D, 2 * M], dt.bfloat16, kind="ExternalInput").ap()
    w2 = nc.dram_tensor("w2", [M, D], dt.bfloat16, kind="ExternalInput").ap()
    ot = nc.dram_tensor("ot", [D, C], dt.float32, kind="ExternalOutput").ap()

    with tile.TileContext(nc) as tc, ExitStack() as ctx:
        wpool = ctx.enter_context(tc.tile_pool(name="w", bufs=1))
        xpool = ctx.enter_context(tc.tile_pool(name="x", bufs=3))
        spool = ctx.enter_context(tc.tile_pool(name="s", bufs=3))
        hpool = ctx.enter_context(tc.tile_pool(name="h", bufs=2))
        opool = ctx.enter_context(tc.tile_pool(name="o", bufs=4))
        pg = ctx.enter_context(tc.tile_pool(name="pg", bufs=2, space="PSUM"))
        pu = ctx.enter_context(tc.tile_pool(name="pu", bufs=2, space="PSUM"))
        po = ctx.enter_context(tc.tile_pool(name="po", bufs=2, space="PSUM"))

        # resident weights: W13 as KD tiles [128, 2M], W2 as KH tiles [128, D]
        w13_t = []
        for k in range(KD):
            t = wpool.tile([128, 2 * M], dt.bfloat16, tag=f"w13_{k}")
            nc.sync.dma_start(t[:], w13[k * 128 : (k + 1) * 128, :])
            w13_t.append(t)
        w2_t = []
        for k in range(KH):
            t = wpool.tile([128, D], dt.bfloat16, tag=f"w2_{k}")
            nc.sync.dma_start(t[:], w2[k * 128 : (k + 1) * 128, :])
            w2_t.append(t)

        nblk = (C + TB - 1) // TB
        for b in range(nblk):
            c0 = b * TB
            n = min(TB, C - c0)
            x_t = []
            for k in range(KD):
                t = xpool.tile([128, n], dt.bfloat16, tag=f"x{k}")
                nc.sync.dma_start(t[:], xt[k * 128 : (k + 1) * 128, c0 : c0 + n])
                x_t.append(t)
            h_t = []
            for j in range(KH):
                g = pg.tile([128, n], dt.float32, tag="pg")
                u = pu.tile([128, n], dt.float32, tag="pu")
                for k in range(KD):
                    nc.tensor.matmul(
                        g[:],
                        w13_t[k][:, j * 128 : (j + 1) * 128],
                        x_t[k][:],
                        start=(k == 0),
                        stop=(k == KD - 1),
                    )
                for k in range(KD):
                    nc.tensor.matmul(
                        u[:],
                        w13_t[k][:, M + j * 128 : M + (j + 1) * 128],
                        x_t[k][:],
                        start=(k == 0),
                        stop=(k == KD - 1),
                    )
                gs = spool.tile([128, n], dt.float32, tag="gs")
                if use_silu:
                    nc.scalar.activation(
                        gs[:], g[:], mybir.ActivationFunctionType.Silu
                    )
                else:
                    sg = spool.tile([128, n], dt.float32, tag="sg")
                    nc.scalar.activation(
                        sg[:], g[:], mybir.ActivationFunctionType.Sigmoid
                    )
                    nc.vector.tensor_mul(gs[:], g[:], sg[:])
                h = hpool.tile([128, n], dt.bfloat16, tag=f"h{j}")
                nc.vector.tensor_mul(h[:], gs[:], u[:])
                h_t.append(h)
            for d in range(KD):
                p = po.tile([128, n], dt.float32, tag="po")
                for j in range(KH):
                    nc.tensor.matmul(
                        p[:],
                        w2_t[j][:, d * 128 : (d + 1) * 128],
                        h_t[j][:],
                        start=(j == 0),
                        stop=(j == KH - 1),
                    )
                o = opool.tile([128, n], dt.float32, tag="o")
                nc.vector.tensor_copy(o[:], p[:])
                nc.sync.dma_start(ot[d * 128 : (d + 1) * 128, c0 : c0 + n], o[:])

    nc.compile()
    return nc


def _route(xf: np.ndarray, moe_router: np.ndarray):
    """Top-2 routing on host. Returns per-expert (rows, weights)."""
    logits = xf @ moe_router                      # [T, E] f32
    top1 = np.argmax(logits, axis=1)
    tmp = logits.copy()
    tmp[np.arange(T), top1] = -np.inf
    top2 = np.argmax(tmp, axis=1)
    l1 = logits[np.arange(T), top1]
    l2 = logits[np.arange(T), top2]
    mx = np.maximum(l1, l2)
    e1 = np.exp(l1 - mx)
    e2 = np.exp(l2 - mx)
    s = e1 + e2
    w1 = (e1 / s).astype(np.float32)
    w2 = (e2 / s).astype(np.float32)
    per_expert = []
    for e in range(E):
        r1 = np.where(top1 == e)[0]
        r2 = np.where(top2 == e)[0]
        rows = np.concatenate([r1, r2])
        wts = np.concatenate([w1[r1], w2[r2]]).astype(np.float32)
        per_expert.append((rows, wts))
    return per_expert


def kernel(x, moe_router, moe_w13, moe_w2, _trace=False, _trace_kwargs=None):
    global last_results
    xf = np.ascontiguousarray(x.reshape(T, D).astype(np.float32))
    per_expert = _route(xf, np.asarray(moe_router, dtype=np.float32))

    cmax = max(len(rows) for rows, _ in per_expert)
    C = ((cmax + 127) // 128) * 128

    nc = _NC_CACHE.get(C)
    if nc is None:
        nc = _build(C)
        _NC_CACHE[C] = nc

    xf_bf = xf.astype(ml_dtypes.bfloat16)
    in_maps = []
    for e in range(E):
        rows, _ = per_expert[e]
        xg = np.zeros((C, D), dtype=ml_dtypes.bfloat16)
        xg[: len(rows)] = xf_bf[rows]
        in_maps.append(
            {
                "xt": np.ascontiguousarray(xg.T),
                "w13": np.ascontiguousarray(
                    np.asarray(moe_w13[e]).astype(ml_dtypes.bfloat16)
                ),
                "w2": np.ascontiguousarray(
                    np.asarray(moe_w2[e]).astype(ml_dtypes.bfloat16)
                ),
            }
        )

    res = run_bass_kernel_spmd(
        nc,
        in_maps,
        core_ids=list(range(E)),
        trace=_trace,
        **(_trace_kwargs or {}),
    )
    last_results = res

    out = np.zeros((T, D), dtype=np.float32)
    for e in range(E):
        rows, wts = per_expert[e]
        ote = res.results[e]["ot"]                # [D, C] f32
        out[rows] += ote[:, : len(rows)].T * wts[:, None]
    return out.reshape(B, S, D)


# revision 9
# speedup vs baseline: 1.0384x; 1.0384x over previous
"""Trainium2 Bass kernel: 8-expert top-2 MoE MLP (SwiGLU), expert-parallel on 8 cores.

Strategy (per sharding hint, expert-parallel):
  - Host: router matmul + top-2 + softmax weights (67 MFLOP — negligible),
    gather each expert's tokens into a zero-padded capacity-C buffer, staged
    TRANSPOSED ([D, C], bf16) so the device kernel needs no transposes at all.
  - Device (per core = one expert): fused SwiGLU FFN as two chained GEMMs with
    features on partitions and tokens on the moving free dim:
      H'^T[2M, C] = (W13 stationary).T-free x X^T moving  (contract D)
      H^T = silu(gate) * up                               (ACT + DVE)
      O^T[D, C]  = (W2 stationary) x H^T moving           (contract M)
  - Host: weighted scatter-add of the 8 per-expert outputs back to token order.

Weights live in SBUF for the whole kernel (12 MB bf16/core). All matmuls are
bf16 with fp32 PSUM accumulation (rel err ~5e-3 vs fp32 reference).
"""

from contextlib import ExitStack

import ml_dtypes
import numpy as np

import concourse.bass as bass  # noqa: F401  (AP helpers)
import concourse.tile as tile
from concourse import bacc, mybir
from concourse.bass_utils import run_bass_kernel_spmd

# nn_MoEMLP_82617990905863 (hardcoded per contract)
B, S, D = 4, 2048, 1024
T = B * S               # 8192 tokens
E = 8                   # experts == cores
TOPK = 2
M = 2048                # MOE_DIM (w13 = [D, 2M], w2 = [M, D])
TB = 512                # token block = max moving free dim
KD = D // 128           # 8 contraction tiles for X @ W13
KH = M // 128           # 16 contraction tiles for H @ W2

_NC_CACHE: dict[int, object] = {}
last_results = None     # BassKernelResults of the most recent run (for test.py)


def _build(C: int, use_silu: bool = True, out_bf16: bool = True):
    """Build + compile the SPMD per-core graph for capacity C (multiple of 128).

    use_silu=False decomposes silu as g*sigmoid(g) (CoreSim lacks the Silu LUT).

    DMA issue order is the critical path: all transfers drain one HW queue at
    ~300 GB/s in issue order, so x block 0 goes first, then W13 in four
    column-chunks (mm1's j-loop starts after the first 2 MB), then the
    remaining x blocks and W2 hidden behind block-0 compute.
    """
    dt = mybir.dt
    odt = dt.bfloat16 if out_bf16 else dt.float32
    nc = bacc.Bacc(
        "TRN2", target_bir_lowering=False, debug=False, enable_asserts=False
    )
    xt = nc.dram_tensor("xt", [D, C], dt.bfloat16, kind="ExternalInput").ap()
    w13 = nc.dram_tensor("w13", [D, 2 * M], dt.bfloat16, kind="ExternalInput").ap()
    w2 = nc.dram_tensor("w2", [M, D], dt.bfloat16, kind="ExternalInput").ap()
    ot = nc.dram_tensor("ot", [D, C], odt, kind="ExternalOutput").ap()

    nblk = (C + TB - 1) // TB
    NCC = 4                      # w13 column chunks (of 512 gate + 512 up cols)
    CW = M // NCC                # 512 gate cols per chunk

    with tile.TileContext(nc) as tc, ExitStack() as ctx:
        wpool = ctx.enter_context(tc.tile_pool(name="w", bufs=1))
        xpool = ctx.enter_context(tc.tile_pool(name="x", bufs=nblk))
        spool = ctx.enter_context(tc.tile_pool(name="s", bufs=3))
        hpool = ctx.enter_context(tc.tile_pool(name="h", bufs=2))
        opool = ctx.enter_context(tc.tile_pool(name="o", bufs=4))
        pg = ctx.enter_context(tc.tile_pool(name="pg", bufs=2, space="PSUM"))
        pu = ctx.enter_context(tc.tile_pool(name="pu", bufs=2, space="PSUM"))
        po = ctx.enter_context(tc.tile_pool(name="po", bufs=2, space="PSUM"))

        def load_x_block(b):
            c0 = b * TB
            n = min(TB, C - c0)
            ts = []
            for k in range(KD):
                t = xpool.tile([128, n], dt.bfloat16, tag=f"x{k}")
                nc.sync.dma_start(t[:], xt[k * 128 : (k + 1) * 128, c0 : c0 + n])
                ts.append(t)
            return ts

        # 1) x block 0 (0.5 MB) — unblocks the first matmul group
        x_blocks = [load_x_block(0)]

        # 2) W13 in column-chunks: wg[c][k] = gate cols, wu[c][k] = up cols
        wg = [[None] * KD for _ in range(NCC)]
        wu = [[None] * KD for _ in range(NCC)]
        for c in range(NCC):
            for k in range(KD):
                tg = wpool.tile([128, CW], dt.bfloat16, tag=f"wg{c}_{k}")
                nc.sync.dma_start(
                    tg[:], w13[k * 128 : (k + 1) * 128, c * CW : (c + 1) * CW]
                )
                wg[c][k] = tg
                tu = wpool.tile([128, CW], dt.bfloat16, tag=f"wu{c}_{k}")
                nc.sync.dma_start(
                    tu[:],
                    w13[k * 128 : (k + 1) * 128, M + c * CW : M + (c + 1) * CW],
                )
                wu[c][k] = tu

        # 3) remaining x blocks + W2, hidden behind block-0 mm1
        for b in range(1, nblk):
            x_blocks.append(load_x_block(b))
        w2_t = []
        for k in range(KH):
            t = wpool.tile([128, D], dt.bfloat16, tag=f"w2_{k}")
            nc.sync.dma_start(t[:], w2[k * 128 : (k + 1) * 128, :])
            w2_t.append(t)

        for b in range(nblk):
            c0 = b * TB
            n = min(TB, C - c0)
            x_t = x_blocks[b]
            h_t = []
            for j in range(KH):
                c, jj = divmod(j, CW // 128)
                g = pg.tile([128, n], dt.float32, tag="pg")
                u = pu.tile([128, n], dt.float32, tag="pu")
                for k in range(KD):
                    nc.tensor.matmul(
                        g[:],
                        wg[c][k][:, jj * 128 : (jj + 1) * 128],
                        x_t[k][:],
                        start=(k == 0),
                        stop=(k == KD - 1),
                    )
                for k in range(KD):
                    nc.tensor.matmul(
                        u[:],
                        wu[c][k][:, jj * 128 : (jj + 1) * 128],
                        x_t[k][:],
                        start=(k == 0),
                        stop=(k == KD - 1),
                    )
                gs = spool.tile([128, n], dt.float32, tag="gs")
                if use_silu:
                    nc.scalar.activation(
                        gs[:], g[:], mybir.ActivationFunctionType.Silu
                    )
                else:
                    sg = spool.tile([128, n], dt.float32, tag="sg")
                    nc.scalar.activation(
                        sg[:], g[:], mybir.ActivationFunctionType.Sigmoid
                    )
                    nc.vector.tensor_mul(gs[:], g[:], sg[:])
                h = hpool.tile([128, n], dt.bfloat16, tag=f"h{j}")
                nc.vector.tensor_mul(h[:], gs[:], u[:])
                h_t.append(h)
            for d in range(KD):
                p = po.tile([128, n], dt.float32, tag="po")
                for j in range(KH):
                    nc.tensor.matmul(
                        p[:],
                        w2_t[j][:, d * 128 : (d + 1) * 128],
                        h_t[j][:],
                        start=(j == 0),
                        stop=(j == KH - 1),
                    )
                o = opool.tile([128, n], odt, tag="o")
                nc.vector.tensor_copy(o[:], p[:])
                nc.sync.dma_start(ot[d * 128 : (d + 1) * 128, c0 : c0 + n], o[:])

    nc.compile()
    return nc


def _route(xf: np.ndarray, moe_router: np.ndarray):
    """Top-2 routing on host. Returns per-expert (rows, weights)."""
    logits = xf @ moe_router                      # [T, E] f32
    top1 = np.argmax(logits, axis=1)
    tmp = logits.copy()
    tmp[np.arange(T), top1] = -np.inf
    top2 = np.argmax(tmp, axis=1)
    l1 = logits[np.arange(T), top1]
    l2 = logits[np.arange(T), top2]
    mx = np.maximum(l1, l2)
    e1 = np.exp(l1 - mx)
    e2 = np.exp(l2 - mx)
    s = e1 + e2
    w1 = (e1 / s).astype(np.float32)
    w2 = (e2 / s).astype(np.float32)
    per_expert = []
    for e in range(E):
        r1 = np.where(top1 == e)[0]
        r2 = np.where(top2 == e)[0]
        rows = np.concatenate([r1, r2])
        wts = np.concatenate([w1[r1], w2[r2]]).astype(np.float32)
        per_expert.append((rows, wts))
    return per_expert


def kernel(x, moe_router, moe_w13, moe_w2, _trace=False, _trace_kwargs=None):
    global last_results
    x = np.asarray(x)
    moe_router = np.asarray(moe_router)
    moe_w13 = np.asarray(moe_w13)
    moe_w2 = np.asarray(moe_w2)
    xf = np.ascontiguousarray(x.reshape(T, D).astype(np.float32))
    per_expert = _route(xf, np.asarray(moe_router, dtype=np.float32))

    cmax = max(len(rows) for rows, _ in per_expert)
    C = ((cmax + 127) // 128) * 128

    nc = _NC_CACHE.get(C)
    if nc is None:
        nc = _build(C)
        _NC_CACHE[C] = nc

    xf_bf = xf.astype(ml_dtypes.bfloat16)
    in_maps = []
    for e in range(E):
        rows, _ = per_expert[e]
        xg = np.zeros((C, D), dtype=ml_dtypes.bfloat16)
        xg[: len(rows)] = xf_bf[rows]
        in_maps.append(
            {
                "xt": np.ascontiguousarray(xg.T),
                "w13": np.ascontiguousarray(
                    np.asarray(moe_w13[e]).astype(ml_dtypes.bfloat16)
                ),
                "w2": np.ascontiguousarray(
                    np.asarray(moe_w2[e]).astype(ml_dtypes.bfloat16)
                ),
            }
        )

    res = run_bass_kernel_spmd(
        nc,
        in_maps,
        core_ids=list(range(E)),
        trace=_trace,
        **(_trace_kwargs or {}),
    )
    last_results = res

    out = np.zeros((T, D), dtype=np.float32)
    for e in range(E):
        rows, wts = per_expert[e]
        ote = res.results[e]["ot"]                # [D, C] bf16 (or f32)
        out[rows] += ote[:, : len(rows)].T.astype(np.float32) * wts[:, None]
    return out.reshape(B, S, D)


# revision 13
# speedup vs baseline: 1.0807x; 1.0408x over previous
"""Trainium2 Bass kernel: 8-expert top-2 MoE MLP (SwiGLU), expert-parallel on 8 cores.

Strategy (per sharding hint, expert-parallel):
  - Host: router matmul + top-2 + softmax weights (67 MFLOP — negligible),
    gather each expert's tokens into a zero-padded capacity-C buffer, staged
    TRANSPOSED ([D, C], bf16) so the device kernel needs no transposes at all.
  - Device (per core = one expert): fused SwiGLU FFN as two chained GEMMs with
    features on partitions and tokens on the moving free dim:
      H'^T[2M, C] = (W13 stationary).T-free x X^T moving  (contract D)
      H^T = silu(gate) * up                               (ACT + DVE)
      O^T[D, C]  = (W2 stationary) x H^T moving           (contract M)
  - Host: weighted scatter-add of the 8 per-expert outputs back to token order.

Weights live in SBUF for the whole kernel (12 MB bf16/core). All matmuls are
bf16 with fp32 PSUM accumulation (rel err ~5e-3 vs fp32 reference).
"""

from contextlib import ExitStack

import ml_dtypes
import numpy as np

import concourse.bass as bass  # noqa: F401  (AP helpers)
import concourse.tile as tile
from concourse import bacc, mybir
from concourse.bass_utils import run_bass_kernel_spmd

# nn_MoEMLP_82617990905863 (hardcoded per contract)
B, S, D = 4, 2048, 1024
T = B * S               # 8192 tokens
E = 8                   # experts == cores
TOPK = 2
M = 2048                # MOE_DIM (w13 = [D, 2M], w2 = [M, D])
TB = 512                # token block = max moving free dim
KD = D // 128           # 8 contraction tiles for X @ W13
KH = M // 128           # 16 contraction tiles for H @ W2

_NC_CACHE: dict[int, object] = {}
last_results = None     # BassKernelResults of the most recent run (for test.py)


def _build(C: int, use_silu: bool = True, out_bf16: bool = True):
    """Build + compile the SPMD per-core graph for capacity C (multiple of 128).

    use_silu=False decomposes silu as g*sigmoid(g) (CoreSim lacks the Silu LUT).

    DMA issue order is the critical path: all transfers drain one HW queue at
    ~300 GB/s in issue order, so x block 0 goes first, then W13 in four
    column-chunks (mm1's j-loop starts after the first 2 MB), then the
    remaining x blocks and W2 hidden behind block-0 compute.
    """
    dt = mybir.dt
    odt = dt.bfloat16 if out_bf16 else dt.float32
    nc = bacc.Bacc(
        "TRN2", target_bir_lowering=False, debug=False, enable_asserts=False
    )
    # w13 arrives host-interleaved: chunk c occupies cols [c*1024, (c+1)*1024) =
    # [512 gate cols c*512.. | 512 up cols c*512..], so one 2D DMA per (c, k).
    xt = nc.dram_tensor("xt", [D, C], dt.bfloat16, kind="ExternalInput").ap()
    w13 = nc.dram_tensor("w13", [D, 2 * M], dt.bfloat16, kind="ExternalInput").ap()
    w2 = nc.dram_tensor("w2", [M, D], dt.bfloat16, kind="ExternalInput").ap()
    ot = nc.dram_tensor("ot", [D, C], odt, kind="ExternalOutput").ap()

    nblk = (C + TB - 1) // TB
    NCC = 4                      # w13 column chunks
    CW = M // NCC                # 512 gate cols (+512 up cols) per chunk

    with tile.TileContext(nc) as tc, ExitStack() as ctx:
        wpool = ctx.enter_context(tc.tile_pool(name="w", bufs=1))
        xpool = ctx.enter_context(tc.tile_pool(name="x", bufs=1))
        spool = ctx.enter_context(tc.tile_pool(name="s", bufs=3))
        hpool = ctx.enter_context(tc.tile_pool(name="h", bufs=2))
        opool = ctx.enter_context(tc.tile_pool(name="o", bufs=4))
        pg = ctx.enter_context(tc.tile_pool(name="pg", bufs=2, space="PSUM"))
        pu = ctx.enter_context(tc.tile_pool(name="pu", bufs=2, space="PSUM"))
        po = ctx.enter_context(tc.tile_pool(name="po", bufs=2, space="PSUM"))

        # 1) x block 0 (0.5 MB, 8 issues) — unblocks the first matmul group
        x0_t = []
        for k in range(KD):
            t = xpool.tile([128, TB], dt.bfloat16, tag=f"x0_{k}")
            nc.sync.dma_start(t[:], xt[k * 128 : (k + 1) * 128, 0:TB])
            x0_t.append(t)

        # 2) W13 column-chunks: one [128, 1024] DMA per (c, k); chunk 0 lands
        #    ~8us in, and mm1's j-loop consumes chunks slower than they arrive
        wc = [[None] * KD for _ in range(NCC)]
        for c in range(NCC):
            for k in range(KD):
                t = wpool.tile([128, 2 * CW], dt.bfloat16, tag=f"wc{c}_{k}")
                nc.sync.dma_start(
                    t[:],
                    w13[k * 128 : (k + 1) * 128, c * 2 * CW : (c + 1) * 2 * CW],
                )
                wc[c][k] = t

        # 3) rest of x (needed from block 1, ~80us) then W2 (needed ~75us)
        xr_t = []
        for k in range(KD):
            t = xpool.tile([128, C - TB], dt.bfloat16, tag=f"xr_{k}")
            nc.sync.dma_start(t[:], xt[k * 128 : (k + 1) * 128, TB:C])
            xr_t.append(t)
        w2_t = []
        for k in range(KH):
            t = wpool.tile([128, D], dt.bfloat16, tag=f"w2_{k}")
            nc.sync.dma_start(t[:], w2[k * 128 : (k + 1) * 128, :])
            w2_t.append(t)

        for b in range(nblk):
            c0 = b * TB
            n = min(TB, C - c0)
            if b == 0:
                x_t = [t[:] for t in x0_t]
            else:
                x_t = [t[:, c0 - TB : c0 - TB + n] for t in xr_t]
            h_t = []
            for j in range(KH):
                c, jj = divmod(j, CW // 128)
                g = pg.tile([128, n], dt.float32, tag="pg")
                u = pu.tile([128, n], dt.float32, tag="pu")
                for k in range(KD):
                    nc.tensor.matmul(
                        g[:],
                        wc[c][k][:, jj * 128 : (jj + 1) * 128],
                        x_t[k],
                        start=(k == 0),
                        stop=(k == KD - 1),
                    )
                for k in range(KD):
                    nc.tensor.matmul(
                        u[:],
                        wc[c][k][:, CW + jj * 128 : CW + (jj + 1) * 128],
                        x_t[k],
                        start=(k == 0),
                        stop=(k == KD - 1),
                    )
                gs = spool.tile([128, n], dt.float32, tag="gs")
                if use_silu:
                    nc.scalar.activation(
                        gs[:], g[:], mybir.ActivationFunctionType.Silu
                    )
                else:
                    sg = spool.tile([128, n], dt.float32, tag="sg")
                    nc.scalar.activation(
                        sg[:], g[:], mybir.ActivationFunctionType.Sigmoid
                    )
                    nc.vector.tensor_mul(gs[:], g[:], sg[:])
                h = hpool.tile([128, n], dt.bfloat16, tag=f"h{j}")
                nc.vector.tensor_mul(h[:], gs[:], u[:])
                h_t.append(h)
            for d in range(KD):
                p = po.tile([128, n], dt.float32, tag="po")
                for j in range(KH):
                    nc.tensor.matmul(
                        p[:],
                        w2_t[j][:, d * 128 : (d + 1) * 128],
                        h_t[j][:],
                        start=(j == 0),
                        stop=(j == KH - 1),
                    )
                o = opool.tile([128, n], odt, tag="o")
                nc.vector.tensor_copy(o[:], p[:])
                nc.sync.dma_start(ot[d * 128 : (d + 1) * 128, c0 : c0 + n], o[:])

    nc.compile()
    return nc


def _interleave_w13(w13_bf: np.ndarray) -> np.ndarray:
    """Reorder [D, 2M] gate|up columns into chunks [g_c | u_c] of 512 each."""
    return np.concatenate(
        [
            np.concatenate(
                [
                    w13_bf[:, c * 512 : (c + 1) * 512],
                    w13_bf[:, M + c * 512 : M + (c + 1) * 512],
                ],
                axis=1,
            )
            for c in range(4)
        ],
        axis=1,
    )


def _route(xf: np.ndarray, moe_router: np.ndarray):
    """Top-2 routing on host. Returns per-expert (rows, weights)."""
    logits = xf @ moe_router                      # [T, E] f32
    top1 = np.argmax(logits, axis=1)
    tmp = logits.copy()
    tmp[np.arange(T), top1] = -np.inf
    top2 = np.argmax(tmp, axis=1)
    l1 = logits[np.arange(T), top1]
    l2 = logits[np.arange(T), top2]
    mx = np.maximum(l1, l2)
    e1 = np.exp(l1 - mx)
    e2 = np.exp(l2 - mx)
    s = e1 + e2
    w1 = (e1 / s).astype(np.float32)
    w2 = (e2 / s).astype(np.float32)
    per_expert = []
    for e in range(E):
        r1 = np.where(top1 == e)[0]
        r2 = np.where(top2 == e)[0]
        rows = np.concatenate([r1, r2])
        wts = np.concatenate([w1[r1], w2[r2]]).astype(np.float32)
        per_expert.append((rows, wts))
    return per_expert


def kernel(x, moe_router, moe_w13, moe_w2, _trace=False, _trace_kwargs=None):
    global last_results
    x = np.asarray(x)
    moe_router = np.asarray(moe_router)
    moe_w13 = np.asarray(moe_w13)
    moe_w2 = np.asarray(moe_w2)
    xf = np.ascontiguousarray(x.reshape(T, D).astype(np.float32))
    per_expert = _route(xf, np.asarray(moe_router, dtype=np.float32))

    cmax = max(len(rows) for rows, _ in per_expert)
    C = ((cmax + 127) // 128) * 128

    nc = _NC_CACHE.get(C)
    if nc is None:
        nc = _build(C)
        _NC_CACHE[C] = nc

    xf_bf = xf.astype(ml_dtypes.bfloat16)
    in_maps = []
    for e in range(E):
        rows, _ = per_expert[e]
        xg = np.zeros((C, D), dtype=ml_dtypes.bfloat16)
        xg[: len(rows)] = xf_bf[rows]
        w13_il = _interleave_w13(np.asarray(moe_w13[e]).astype(ml_dtypes.bfloat16))
        in_maps.append(
            {
                "xt": np.ascontiguousarray(xg.T),
                "w13": np.ascontiguousarray(w13_il),
                "w2": np.ascontiguousarray(
                    np.asarray(moe_w2[e]).astype(ml_dtypes.bfloat16)
                ),
            }
        )

    res = run_bass_kernel_spmd(
        nc,
        in_maps,
        core_ids=list(range(E)),
        trace=_trace,
        **(_trace_kwargs or {}),
    )
    last_results = res

    out = np.zeros((T, D), dtype=np.float32)
    for e in range(E):
        rows, wts = per_expert[e]
        ote = res.results[e]["ot"]                # [D, C] bf16 (or f32)
        out[rows] += ote[:, : len(rows)].T.astype(np.float32) * wts[:, None]
    return out.reshape(B, S, D)


# revision 14
# speedup vs baseline: 1.0838x; 1.0028x over previous
"""Trainium2 Bass kernel: 8-expert top-2 MoE MLP (SwiGLU), expert-parallel on 8 cores.

Strategy (per sharding hint, expert-parallel):
  - Host: router matmul + top-2 + softmax weights (67 MFLOP — negligible),
    gather each expert's tokens into a zero-padded capacity-C buffer, staged
    TRANSPOSED ([D, C], bf16) so the device kernel needs no transposes at all.
  - Device (per core = one expert): fused SwiGLU FFN as two chained GEMMs with
    features on partitions and tokens on the moving free dim:
      H'^T[2M, C] = (W13 stationary).T-free x X^T moving  (contract D)
      H^T = silu(gate) * up                               (ACT + DVE)
      O^T[D, C]  = (W2 stationary) x H^T moving           (contract M)
  - Host: weighted scatter-add of the 8 per-expert outputs back to token order.

Weights live in SBUF for the whole kernel (12 MB bf16/core). All matmuls are
bf16 with fp32 PSUM accumulation (rel err ~5e-3 vs fp32 reference).
"""

from contextlib import ExitStack

import ml_dtypes
import numpy as np

import concourse.bass as bass  # noqa: F401  (AP helpers)
import concourse.tile as tile
from concourse import bacc, mybir
from concourse.bass_utils import run_bass_kernel_spmd

# nn_MoEMLP_82617990905863 (hardcoded per contract)
B, S, D = 4, 2048, 1024
T = B * S               # 8192 tokens
E = 8                   # experts == cores
TOPK = 2
M = 2048                # MOE_DIM (w13 = [D, 2M], w2 = [M, D])
TB = 512                # token block = max moving free dim
KD = D // 128           # 8 contraction tiles for X @ W13
KH = M // 128           # 16 contraction tiles for H @ W2

_NC_CACHE: dict[int, object] = {}
last_results = None     # BassKernelResults of the most recent run (for test.py)


def _build(C: int, use_silu: bool = True, out_bf16: bool = True):
    """Build + compile the SPMD per-core graph for capacity C (multiple of 128).

    use_silu=False decomposes silu as g*sigmoid(g) (CoreSim lacks the Silu LUT).

    DMA issue order is the critical path: all transfers drain one HW queue at
    ~300 GB/s in issue order, so x block 0 goes first, then W13 in four
    column-chunks (mm1's j-loop starts after the first 2 MB), then the
    remaining x blocks and W2 hidden behind block-0 compute.
    """
    dt = mybir.dt
    odt = dt.bfloat16 if out_bf16 else dt.float32
    nc = bacc.Bacc(
        "TRN2", target_bir_lowering=False, debug=False, enable_asserts=False
    )
    # w13 arrives host-interleaved: chunk c occupies cols [c*1024, (c+1)*1024) =
    # [512 gate cols c*512.. | 512 up cols c*512..], so one 2D DMA per (c, k).
    xt = nc.dram_tensor("xt", [D, C], dt.bfloat16, kind="ExternalInput").ap()
    w13 = nc.dram_tensor("w13", [D, 2 * M], dt.bfloat16, kind="ExternalInput").ap()
    w2 = nc.dram_tensor("w2", [M, D], dt.bfloat16, kind="ExternalInput").ap()
    ot = nc.dram_tensor("ot", [D, C], odt, kind="ExternalOutput").ap()

    nblk = (C + TB - 1) // TB
    NCC = 4                      # w13 column chunks
    CW = M // NCC                # 512 gate cols (+512 up cols) per chunk

    with tile.TileContext(nc) as tc, ExitStack() as ctx:
        wpool = ctx.enter_context(tc.tile_pool(name="w", bufs=1))
        xpool = ctx.enter_context(tc.tile_pool(name="x", bufs=1))
        spool = ctx.enter_context(tc.tile_pool(name="s", bufs=3))
        hpool = ctx.enter_context(tc.tile_pool(name="h", bufs=2))
        opool = ctx.enter_context(tc.tile_pool(name="o", bufs=4))
        pg = ctx.enter_context(tc.tile_pool(name="pg", bufs=2, space="PSUM"))
        pu = ctx.enter_context(tc.tile_pool(name="pu", bufs=2, space="PSUM"))
        po = ctx.enter_context(tc.tile_pool(name="po", bufs=2, space="PSUM"))

        # 1) x block 0 (0.5 MB, 8 issues) — unblocks the first matmul group
        x0_t = []
        for k in range(KD):
            t = xpool.tile([128, TB], dt.bfloat16, tag=f"x0_{k}")
            nc.sync.dma_start(t[:], xt[k * 128 : (k + 1) * 128, 0:TB])
            x0_t.append(t)

        # 2) W13 column-chunks: one [128, 1024] DMA per (c, k); chunk 0 lands
        #    ~8us in, and mm1's j-loop consumes chunks slower than they arrive
        wc = [[None] * KD for _ in range(NCC)]
        for c in range(NCC):
            for k in range(KD):
                t = wpool.tile([128, 2 * CW], dt.bfloat16, tag=f"wc{c}_{k}")
                nc.sync.dma_start(
                    t[:],
                    w13[k * 128 : (k + 1) * 128, c * 2 * CW : (c + 1) * 2 * CW],
                )
                wc[c][k] = t

        # 3) rest of x (needed from block 1, ~80us) then W2 (needed ~75us)
        xr_t = []
        for k in range(KD):
            t = xpool.tile([128, C - TB], dt.bfloat16, tag=f"xr_{k}")
            nc.sync.dma_start(t[:], xt[k * 128 : (k + 1) * 128, TB:C])
            xr_t.append(t)
        w2_t = []
        for k in range(KH):
            t = wpool.tile([128, D], dt.bfloat16, tag=f"w2_{k}")
            nc.sync.dma_start(t[:], w2[k * 128 : (k + 1) * 128, :])
            w2_t.append(t)

        for b in range(nblk):
            c0 = b * TB
            n = min(TB, C - c0)
            if b == 0:
                x_t = [t[:] for t in x0_t]
            else:
                x_t = [t[:, c0 - TB : c0 - TB + n] for t in xr_t]
            h_t = []
            for j in range(KH):
                c, jj = divmod(j, CW // 128)
                g = pg.tile([128, n], dt.float32, tag="pg")
                u = pu.tile([128, n], dt.float32, tag="pu")
                for k in range(KD):
                    nc.tensor.matmul(
                        g[:],
                        wc[c][k][:, jj * 128 : (jj + 1) * 128],
                        x_t[k],
                        start=(k == 0),
                        stop=(k == KD - 1),
                    )
                for k in range(KD):
                    nc.tensor.matmul(
                        u[:],
                        wc[c][k][:, CW + jj * 128 : CW + (jj + 1) * 128],
                        x_t[k],
                        start=(k == 0),
                        stop=(k == KD - 1),
                    )
                gs = spool.tile([128, n], dt.float32, tag="gs")
                if use_silu:
                    nc.scalar.activation(
                        gs[:], g[:], mybir.ActivationFunctionType.Silu
                    )
                else:
                    sg = spool.tile([128, n], dt.float32, tag="sg")
                    nc.scalar.activation(
                        sg[:], g[:], mybir.ActivationFunctionType.Sigmoid
                    )
                    nc.vector.tensor_mul(gs[:], g[:], sg[:])
                h = hpool.tile([128, n], dt.bfloat16, tag=f"h{j}")
                nc.vector.tensor_mul(h[:], gs[:], u[:])
                h_t.append(h)
            for d in range(KD):
                p = po.tile([128, n], dt.float32, tag="po")
                for j in range(KH):
                    nc.tensor.matmul(
                        p[:],
                        w2_t[j][:, d * 128 : (d + 1) * 128],
                        h_t[j][:],
                        start=(j == 0),
                        stop=(j == KH - 1),
                    )
                o = opool.tile([128, n], odt, tag="o")
                nc.vector.tensor_copy(o[:], p[:])
                nc.sync.dma_start(ot[d * 128 : (d + 1) * 128, c0 : c0 + n], o[:])

    nc.compile()
    return nc


def _interleave_w13(w13_bf: np.ndarray) -> np.ndarray:
    """Reorder [D, 2M] gate|up columns into chunks [g_c | u_c] of 512 each."""
    return np.concatenate(
        [
            np.concatenate(
                [
                    w13_bf[:, c * 512 : (c + 1) * 512],
                    w13_bf[:, M + c * 512 : M + (c + 1) * 512],
                ],
                axis=1,
            )
            for c in range(4)
        ],
        axis=1,
    )


def _route(xf: np.ndarray, moe_router: np.ndarray):
    """Top-2 routing on host. Returns per-expert (rows, weights)."""
    logits = xf @ moe_router                      # [T, E] f32
    top1 = np.argmax(logits, axis=1)
    tmp = logits.copy()
    tmp[np.arange(T), top1] = -np.inf
    top2 = np.argmax(tmp, axis=1)
    l1 = logits[np.arange(T), top1]
    l2 = logits[np.arange(T), top2]
    mx = np.maximum(l1, l2)
    e1 = np.exp(l1 - mx)
    e2 = np.exp(l2 - mx)
    s = e1 + e2
    w1 = (e1 / s).astype(np.float32)
    w2 = (e2 / s).astype(np.float32)
    per_expert = []
    for e in range(E):
        r1 = np.where(top1 == e)[0]
        r2 = np.where(top2 == e)[0]
        rows = np.concatenate([r1, r2])
        wts = np.concatenate([w1[r1], w2[r2]]).astype(np.float32)
        per_expert.append((rows, wts))
    return per_expert


def kernel(x, moe_router, moe_w13, moe_w2, _trace=False, _trace_kwargs=None):
    global last_results
    x = np.asarray(x)
    moe_router = np.asarray(moe_router)
    moe_w13 = np.asarray(moe_w13)
    moe_w2 = np.asarray(moe_w2)
    xf = np.ascontiguousarray(x.reshape(T, D).astype(np.float32))
    per_expert = _route(xf, np.asarray(moe_router, dtype=np.float32))

    cmax = max(len(rows) for rows, _ in per_expert)
    C = cmax + (cmax & 1)       # even, else exact (padding is pure overhead)

    nc = _NC_CACHE.get(C)
    if nc is None:
        nc = _build(C)
        _NC_CACHE[C] = nc

    xf_bf = xf.astype(ml_dtypes.bfloat16)
    in_maps = []
    for e in range(E):
        rows, _ = per_expert[e]
        xg = np.zeros((C, D), dtype=ml_dtypes.bfloat16)
        xg[: len(rows)] = xf_bf[rows]
        w13_il = _interleave_w13(np.asarray(moe_w13[e]).astype(ml_dtypes.bfloat16))
        in_maps.append(
            {
                "xt": np.ascontiguousarray(xg.T),
                "w13": np.ascontiguousarray(w13_il),
                "w2": np.ascontiguousarray(
                    np.asarray(moe_w2[e]).astype(ml_dtypes.bfloat16)
                ),
            }
        )

    res = run_bass_kernel_spmd(
        nc,
        in_maps,
        core_ids=list(range(E)),
        trace=_trace,
        **(_trace_kwargs or {}),
    )
    last_results = res

    out = np.zeros((T, D), dtype=np.float32)
    for e in range(E):
        rows, wts = per_expert[e]
        ote = res.results[e]["ot"]                # [D, C] bf16 (or f32)
        out[rows] += ote[:, : len(rows)].T.astype(np.float32) * wts[:, None]
    return out.reshape(B, S, D)


# revision 18
# speedup vs baseline: 1.0941x; 1.0096x over previous
"""Trainium2 Bass kernel: 8-expert top-2 MoE MLP (SwiGLU), expert-parallel on 8 cores.

Strategy (per sharding hint, expert-parallel):
  - Host: router matmul + top-2 + softmax weights (67 MFLOP — negligible),
    gather each expert's tokens into a zero-padded capacity-C buffer, staged
    TRANSPOSED and chunk-major so every device DMA is one contiguous 2D copy.
  - Device (per core = one expert): fused SwiGLU FFN as two chained GEMMs with
    features on partitions and tokens on the moving free dim:
      H'^T[2M, C] = (W13 stationary).T-free x X^T moving  (contract D)
      H^T = silu(gate) * up                               (ACT + DVE)
      O^T[D, C]  = (W2 stationary) x H^T moving           (contract M)
  - Host: weighted scatter-add of the 8 per-expert outputs back to token order.

Weights live in SBUF for the whole kernel (12 MB bf16/core). All matmuls are
bf16 with fp32 PSUM accumulation (rel err ~5e-3 vs fp32 reference).

DMA issue order/count is the critical path (~0.6us per dma_start on the issuing
engine, transfers drain in issue order at ~300 GB/s): x block 0 (1 DMA), then
W13 in 8 chunk-major DMAs (the first 1 MB chunk unblocks mm1's j-loop), then
the remaining x blocks and W2 hidden behind block-0 compute.
"""

from contextlib import ExitStack

import ml_dtypes
import numpy as np

import concourse.bass as bass  # noqa: F401  (AP helpers)
import concourse.tile as tile
from concourse import bacc, mybir
from concourse.bass_utils import run_bass_kernel_spmd

# nn_MoEMLP_82617990905863 (hardcoded per contract)
B, S, D = 4, 2048, 1024
T = B * S               # 8192 tokens
E = 8                   # experts == cores
TOPK = 2
M = 2048                # MOE_DIM (w13 = [D, 2M], w2 = [M, D])
TB = 512                # token block = max moving free dim
KD = D // 128           # 8 contraction tiles for X @ W13
KH = M // 128           # 16 contraction tiles for H @ W2
NCC = 8                 # w13 chunks; chunk c covers hidden cols [c*256,(c+1)*256)
CG = M // NCC           # 256 gate (and 256 up) cols per chunk

_NC_CACHE: dict[int, object] = {}
last_results = None     # BassKernelResults of the most recent run (for test.py)


def _nblocks(C: int) -> int:
    return (C + TB - 1) // TB


def _build(C: int, use_silu: bool = True, out_bf16: bool = True):
    """Build + compile the SPMD per-core graph for capacity C (even).

    use_silu=False decomposes silu as g*sigmoid(g) (CoreSim lacks the Silu LUT).
    """
    dt = mybir.dt
    odt = dt.bfloat16 if out_bf16 else dt.float32
    nc = bacc.Bacc(
        "TRN2", target_bir_lowering=False, debug=False, enable_asserts=False
    )
    nblk = _nblocks(C)
    # chunk-major host layouts — each DMA below is a contiguous [128, W] copy:
    #   xt : [p, block, k, tok]     w13: [p, c, k, g256|u256]   w2: [p, k, d]
    xt = nc.dram_tensor("xt", [128, KD * C], dt.bfloat16, kind="ExternalInput").ap()
    w13 = nc.dram_tensor(
        "w13", [128, NCC * KD * 2 * CG], dt.bfloat16, kind="ExternalInput"
    ).ap()
    w2 = nc.dram_tensor("w2", [128, KH * D], dt.bfloat16, kind="ExternalInput").ap()
    ot = nc.dram_tensor("ot", [D, C], odt, kind="ExternalOutput").ap()

    with tile.TileContext(nc) as tc, ExitStack() as ctx:
        wpool = ctx.enter_context(tc.tile_pool(name="w", bufs=1))
        xpool = ctx.enter_context(tc.tile_pool(name="x", bufs=1))
        spool = ctx.enter_context(tc.tile_pool(name="s", bufs=3))
        hpool = ctx.enter_context(tc.tile_pool(name="h", bufs=2))
        opool = ctx.enter_context(tc.tile_pool(name="o", bufs=4))
        pg = ctx.enter_context(tc.tile_pool(name="pg", bufs=3, space="PSUM"))
        pu = ctx.enter_context(tc.tile_pool(name="pu", bufs=3, space="PSUM"))
        po = ctx.enter_context(tc.tile_pool(name="po", bufs=2, space="PSUM"))

        def x_off(b):
            return KD * b * TB

        # 1) x block 0 — one 0.5 MB DMA unblocks the first matmul group
        xb = [None] * nblk
        n0 = min(TB, C)
        xb0_tile = xpool.tile([128, KD * n0], dt.bfloat16, tag="xb0")
        xb[0] = xb0_tile
        nc.sync.dma_start(xb0_tile[:], xt[:, 0 : KD * n0])

        # 2) W13 chunk-major: one 1 MB DMA per chunk; chunk 0 lands ~12us in
        #    and mm1 consumes chunks (2 j's = 6.8us) slower than they arrive
        wt = []
        for c in range(NCC):
            t = wpool.tile([128, KD * 2 * CG], dt.bfloat16, tag=f"wc{c}")
            nc.sync.dma_start(
                t[:], w13[:, c * KD * 2 * CG : (c + 1) * KD * 2 * CG]
            )
            wt.append(t)

        # 3) rest of x (needed from ~80us) then W2 (needed ~75us)
        for b in range(1, nblk):
            n = min(TB, C - b * TB)
            xb_tile = xpool.tile([128, KD * n], dt.bfloat16, tag=f"xb{b}")
            xb[b] = xb_tile
            nc.sync.dma_start(xb_tile[:], xt[:, x_off(b) : x_off(b) + KD * n])
        w2t = wpool.tile([128, KH * D], dt.bfloat16, tag="w2")
        nc.sync.dma_start(w2t[:], w2[:, :])

        for b in range(nblk):
            c0 = b * TB
            n = min(TB, C - c0)
            h_t = []
            for j in range(KH):
                c, jj = divmod(j, CG // 128)
                g = pg.tile([128, n], dt.float32, tag="pg")
                u = pu.tile([128, n], dt.float32, tag="pu")
                for k in range(KD):
                    nc.tensor.matmul(
                        g[:],
                        wt[c][:, k * 2 * CG + jj * 128 : k * 2 * CG + (jj + 1) * 128],
                        xb[b][:, k * n : (k + 1) * n],
                        start=(k == 0),
                        stop=(k == KD - 1),
                    )
                for k in range(KD):
                    nc.tensor.matmul(
                        u[:],
                        wt[c][
                            :,
                            k * 2 * CG + CG + jj * 128 : k * 2 * CG
                            + CG
                            + (jj + 1) * 128,
                        ],
                        xb[b][:, k * n : (k + 1) * n],
                        start=(k == 0),
                        stop=(k == KD - 1),
                    )
                gs = spool.tile([128, n], dt.float32, tag="gs")
                if use_silu:
                    nc.scalar.activation(
                        gs[:], g[:], mybir.ActivationFunctionType.Silu
                    )
                else:
                    sg = spool.tile([128, n], dt.float32, tag="sg")
                    nc.scalar.activation(
                        sg[:], g[:], mybir.ActivationFunctionType.Sigmoid
                    )
                    nc.vector.tensor_mul(gs[:], g[:], sg[:])
                h = hpool.tile([128, n], dt.bfloat16, tag=f"h{j}")
                nc.vector.tensor_mul(h[:], gs[:], u[:])
                h_t.append(h)
            for d in range(KD):
                p = po.tile([128, n], dt.float32, tag="po")
                for j in range(KH):
                    nc.tensor.matmul(
                        p[:],
                        w2t[:, j * D + d * 128 : j * D + (d + 1) * 128],
                        h_t[j][:],
                        start=(j == 0),
                        stop=(j == KH - 1),
                    )
                o = opool.tile([128, n], odt, tag="o")
                nc.vector.tensor_copy(o[:], p[:])
                nc.sync.dma_start(ot[d * 128 : (d + 1) * 128, c0 : c0 + n], o[:])

    nc.compile()
    return nc


def _stage_x(xg: np.ndarray) -> np.ndarray:
    """[C, D] gathered tokens -> [128, block-major (b, k, tok)] bf16."""
    C = xg.shape[0]
    a = np.ascontiguousarray(xg.T).reshape(KD, 128, C)       # [k, p, tok]
    blocks = []
    for b in range(_nblocks(C)):
        c0 = b * TB
        n = min(TB, C - c0)
        blocks.append(a[:, :, c0 : c0 + n].transpose(1, 0, 2).reshape(128, KD * n))
    return np.ascontiguousarray(np.concatenate(blocks, axis=1))


def _stage_w13(w: np.ndarray) -> np.ndarray:
    """[D, 2M] gate|up -> [128, chunk-major (c, k, g256|u256)] bf16."""
    g = w[:, :M].reshape(KD, 128, NCC, CG)                   # [k, p, c, i]
    u = w[:, M:].reshape(KD, 128, NCC, CG)
    a = np.concatenate([g, u], axis=3)                       # [k, p, c, 2CG]
    return np.ascontiguousarray(
        a.transpose(1, 2, 0, 3).reshape(128, NCC * KD * 2 * CG)
    )


def _stage_w2(w: np.ndarray) -> np.ndarray:
    """[M, D] -> [128, (k, d)] bf16."""
    return np.ascontiguousarray(
        w.reshape(KH, 128, D).transpose(1, 0, 2).reshape(128, KH * D)
    )


def _route(xf: np.ndarray, moe_router: np.ndarray):
    """Top-2 routing on host. Returns per-expert (rows, weights)."""
    logits = xf @ moe_router                      # [T, E] f32
    top1 = np.argmax(logits, axis=1)
    tmp = logits.copy()
    tmp[np.arange(T), top1] = -np.inf
    top2 = np.argmax(tmp, axis=1)
    l1 = logits[np.arange(T), top1]
    l2 = logits[np.arange(T), top2]
    mx = np.maximum(l1, l2)
    e1 = np.exp(l1 - mx)
    e2 = np.exp(l2 - mx)
    s = e1 + e2
    w1 = (e1 / s).astype(np.float32)
    w2 = (e2 / s).astype(np.float32)
    per_expert = []
    for e in range(E):
        r1 = np.where(top1 == e)[0]
        r2 = np.where(top2 == e)[0]
        rows = np.concatenate([r1, r2])
        wts = np.concatenate([w1[r1], w2[r2]]).astype(np.float32)
        per_expert.append((rows, wts))
    return per_expert


def kernel(x, moe_router, moe_w13, moe_w2, _trace=False, _trace_kwargs=None):
    global last_results
    x = np.asarray(x)
    moe_router = np.asarray(moe_router)
    moe_w13 = np.asarray(moe_w13)
    moe_w2 = np.asarray(moe_w2)
    xf = np.ascontiguousarray(x.reshape(T, D).astype(np.float32))
    per_expert = _route(xf, np.asarray(moe_router, dtype=np.float32))

    cmax = max(len(rows) for rows, _ in per_expert)
    C = cmax + (cmax & 1)       # even, else exact (padding is pure overhead)
    C = max(C, 2 * TB)          # keep the block-0 / rest split well-formed

    nc = _NC_CACHE.get(C)
    if nc is None:
        nc = _build(C)
        _NC_CACHE[C] = nc

    xf_bf = xf.astype(ml_dtypes.bfloat16)
    in_maps = []
    for e in range(E):
        rows, _ = per_expert[e]
        xg = np.zeros((C, D), dtype=ml_dtypes.bfloat16)
        xg[: len(rows)] = xf_bf[rows]
        in_maps.append(
            {
                "xt": _stage_x(xg),
                "w13": _stage_w13(
                    np.asarray(moe_w13[e]).astype(ml_dtypes.bfloat16)
                ),
                "w2": _stage_w2(np.asarray(moe_w2[e]).astype(ml_dtypes.bfloat16)),
            }
        )

    res = run_bass_kernel_spmd(
        nc,
        in_maps,
        core_ids=list(range(E)),
        trace=_trace,
        **(_trace_kwargs or {}),
    )
    last_results = res

    out = np.zeros((T, D), dtype=np.float32)
    for e in range(E):
        rows, wts = per_expert[e]
        ote = res.results[e]["ot"]                # [D, C] bf16 (or f32)
        out[rows] += ote[:, : len(rows)].T.astype(np.float32) * wts[:, None]
    return out.reshape(B, S, D)


# revision 23
# speedup vs baseline: 1.1044x; 1.0093x over previous
"""Trainium2 Bass kernel: 8-expert top-2 MoE MLP (SwiGLU), expert-parallel on 8 cores.

Strategy (per sharding hint, expert-parallel):
  - Host: router matmul + top-2 + softmax weights (67 MFLOP — negligible),
    gather each expert's tokens into a zero-padded capacity-C buffer, staged
    TRANSPOSED and chunk-major so every device DMA is one contiguous 2D copy.
  - Device (per core = one expert): fused SwiGLU FFN as two chained GEMMs with
    features on partitions and tokens on the moving free dim:
      H'^T[2M, C] = (W13 stationary).T-free x X^T moving  (contract D)
      H^T = silu(gate) * up                               (ACT + DVE)
      O^T[D, C]  = (W2 stationary) x H^T moving           (contract M)
  - Host: weighted scatter-add of the 8 per-expert outputs back to token order.

Weights live in SBUF for the whole kernel (12 MB bf16/core). All matmuls are
bf16 with fp32 PSUM accumulation (rel err ~5e-3 vs fp32 reference).

DMA issue order/count is the critical path (~0.6us per dma_start on the issuing
engine, transfers drain in issue order at ~300 GB/s): x block 0 (1 DMA), then
W13 in 8 chunk-major DMAs (the first 1 MB chunk unblocks mm1's j-loop), then
the remaining x blocks and W2 hidden behind block-0 compute.
"""

from contextlib import ExitStack

import ml_dtypes
import numpy as np

import concourse.bass as bass  # noqa: F401  (AP helpers)
import concourse.tile as tile
from concourse import bacc, mybir
from concourse.bass_utils import run_bass_kernel_spmd

# nn_MoEMLP_82617990905863 (hardcoded per contract)
B, S, D = 4, 2048, 1024
T = B * S               # 8192 tokens
E = 8                   # experts == cores
TOPK = 2
M = 2048                # MOE_DIM (w13 = [D, 2M], w2 = [M, D])
TB = 512                # token block = max moving free dim
KD = D // 128           # 8 contraction tiles for X @ W13
KH = M // 128           # 16 contraction tiles for H @ W2
# w13 chunk schedule: chunk i covers hidden-col tiles W13_CHUNKS[i] (j indices).
# The first two chunks are single-j (0.5 MB) so the first matmul group is
# unblocked after ~1 MB of transfer; the rest are 2-j (1 MB).
W13_CHUNKS = [[0], [1]] + [[j, j + 1] for j in range(2, 16, 2)]

_NC_CACHE: dict[int, object] = {}
last_results = None     # BassKernelResults of the most recent run (for test.py)


def _nblocks(C: int) -> int:
    return (C + TB - 1) // TB


def _build(C: int, use_silu: bool = True, out_bf16: bool = True):
    """Build + compile the SPMD per-core graph for capacity C (even).

    use_silu=False decomposes silu as g*sigmoid(g) (CoreSim lacks the Silu LUT).
    """
    dt = mybir.dt
    odt = dt.bfloat16 if out_bf16 else dt.float32
    nc = bacc.Bacc(
        "TRN2", target_bir_lowering=False, debug=False, enable_asserts=False
    )
    nblk = _nblocks(C)
    # chunk-major host layouts — each DMA below is a contiguous [128, W] copy:
    #   xt : [p, block, k, tok]     w13: [p, chunk, k, g|u]     w2: [p, k, d]
    xt = nc.dram_tensor("xt", [128, KD * C], dt.bfloat16, kind="ExternalInput").ap()
    w13 = nc.dram_tensor(
        "w13", [128, KD * 2 * M], dt.bfloat16, kind="ExternalInput"
    ).ap()
    w2 = nc.dram_tensor("w2", [128, KH * D], dt.bfloat16, kind="ExternalInput").ap()
    ot = nc.dram_tensor("ot", [D, C], odt, kind="ExternalOutput").ap()

    with tile.TileContext(nc) as tc, ExitStack() as ctx:
        wpool = ctx.enter_context(tc.tile_pool(name="w", bufs=1))
        xpool = ctx.enter_context(tc.tile_pool(name="x", bufs=1))
        spool = ctx.enter_context(tc.tile_pool(name="s", bufs=3))
        hpool = ctx.enter_context(tc.tile_pool(name="h", bufs=2))
        opool = ctx.enter_context(tc.tile_pool(name="o", bufs=4))
        pg = ctx.enter_context(tc.tile_pool(name="pg", bufs=3, space="PSUM"))
        pu = ctx.enter_context(tc.tile_pool(name="pu", bufs=3, space="PSUM"))
        po = ctx.enter_context(tc.tile_pool(name="po", bufs=2, space="PSUM"))

        def x_off(b):
            return KD * b * TB

        # 1) x block 0 in two k-halves + w13 chunk 0 interleaved, so the first
        #    matmul group is gated on only ~1 MB of transfer
        xb = [None] * nblk
        n0 = min(TB, C)
        x0a = xpool.tile([128, 4 * n0], dt.bfloat16, tag="x0a")
        nc.sync.dma_start(x0a[:], xt[:, 0 : 4 * n0])

        wt = []
        j_chunk = {}            # j -> (chunk idx, local jj, cgw)
        w13_offs = []
        off = 0
        for ci, js in enumerate(W13_CHUNKS):
            w13_offs.append(off)
            for jj, j in enumerate(js):
                j_chunk[j] = (ci, jj, 128 * len(js))
            off += KD * 2 * 128 * len(js)

        def load_w13_chunk(ci):
            js = W13_CHUNKS[ci]
            cgw = 128 * len(js)
            t = wpool.tile([128, KD * 2 * cgw], dt.bfloat16, tag=f"wc{ci}")
            nc.sync.dma_start(
                t[:], w13[:, w13_offs[ci] : w13_offs[ci] + KD * 2 * cgw]
            )
            return t

        wt.append(load_w13_chunk(0))
        x0b = xpool.tile([128, 4 * n0], dt.bfloat16, tag="x0b")
        nc.sync.dma_start(x0b[:], xt[:, 4 * n0 : 8 * n0])
        for ci in range(1, len(W13_CHUNKS)):
            wt.append(load_w13_chunk(ci))

        # 3) rest of x (needed from ~80us) then W2 (needed ~75us)
        for b in range(1, nblk):
            n = min(TB, C - b * TB)
            xb_tile = xpool.tile([128, KD * n], dt.bfloat16, tag=f"xb{b}")
            xb[b] = xb_tile
            nc.sync.dma_start(xb_tile[:], xt[:, x_off(b) : x_off(b) + KD * n])
        w2t = wpool.tile([128, KH * D], dt.bfloat16, tag="w2")
        nc.sync.dma_start(w2t[:], w2[:, :])

        def x_slice(b, k, n):
            if b == 0:
                if k < 4:
                    return x0a[:, k * n : (k + 1) * n]
                return x0b[:, (k - 4) * n : (k - 3) * n]
            return xb[b][:, k * n : (k + 1) * n]

        for b in range(nblk):
            c0 = b * TB
            n = min(TB, C - c0)
            h_t = []
            for j in range(KH):
                ci, jj, cgw = j_chunk[j]
                g = pg.tile([128, n], dt.float32, tag="pg")
                u = pu.tile([128, n], dt.float32, tag="pu")
                for k in range(KD):
                    nc.tensor.matmul(
                        g[:],
                        wt[ci][
                            :, k * 2 * cgw + jj * 128 : k * 2 * cgw + (jj + 1) * 128
                        ],
                        x_slice(b, k, n),
                        start=(k == 0),
                        stop=(k == KD - 1),
                    )
                for k in range(KD):
                    nc.tensor.matmul(
                        u[:],
                        wt[ci][
                            :,
                            k * 2 * cgw + cgw + jj * 128 : k * 2 * cgw
                            + cgw
                            + (jj + 1) * 128,
                        ],
                        x_slice(b, k, n),
                        start=(k == 0),
                        stop=(k == KD - 1),
                    )
                gs = spool.tile([128, n], dt.float32, tag="gs")
                if use_silu:
                    nc.scalar.activation(
                        gs[:], g[:], mybir.ActivationFunctionType.Silu
                    )
                else:
                    sg = spool.tile([128, n], dt.float32, tag="sg")
                    nc.scalar.activation(
                        sg[:], g[:], mybir.ActivationFunctionType.Sigmoid
                    )
                    nc.vector.tensor_mul(gs[:], g[:], sg[:])
                h = hpool.tile([128, n], dt.bfloat16, tag=f"h{j}")
                nc.vector.tensor_mul(h[:], gs[:], u[:])
                h_t.append(h)
            for d in range(KD):
                p = po.tile([128, n], dt.float32, tag="po")
                for j in range(KH):
                    nc.tensor.matmul(
                        p[:],
                        w2t[:, j * D + d * 128 : j * D + (d + 1) * 128],
                        h_t[j][:],
                        start=(j == 0),
                        stop=(j == KH - 1),
                    )
                o = opool.tile([128, n], odt, tag="o")
                nc.vector.tensor_copy(o[:], p[:])
                nc.sync.dma_start(ot[d * 128 : (d + 1) * 128, c0 : c0 + n], o[:])

    nc.compile()
    return nc


def _stage_x(xg: np.ndarray) -> np.ndarray:
    """[C, D] gathered tokens -> [128, block-major (b, k, tok)] bf16."""
    C = xg.shape[0]
    a = np.ascontiguousarray(xg.T).reshape(KD, 128, C)       # [k, p, tok]
    blocks = []
    for b in range(_nblocks(C)):
        c0 = b * TB
        n = min(TB, C - c0)
        blocks.append(a[:, :, c0 : c0 + n].transpose(1, 0, 2).reshape(128, KD * n))
    return np.ascontiguousarray(np.concatenate(blocks, axis=1))


def _stage_w13(w: np.ndarray) -> np.ndarray:
    """[D, 2M] gate|up -> [128, chunk-major (chunk, k, g|u)] bf16."""
    parts = []
    for js in W13_CHUNKS:
        cgw = 128 * len(js)
        cols_g = np.concatenate([w[:, j * 128 : (j + 1) * 128] for j in js], axis=1)
        cols_u = np.concatenate(
            [w[:, M + j * 128 : M + (j + 1) * 128] for j in js], axis=1
        )
        a = np.concatenate([cols_g, cols_u], axis=1)         # [D, 2cgw]
        parts.append(
            a.reshape(KD, 128, 2 * cgw).transpose(1, 0, 2).reshape(128, KD * 2 * cgw)
        )
    return np.ascontiguousarray(np.concatenate(parts, axis=1))


def _stage_w2(w: np.ndarray) -> np.ndarray:
    """[M, D] -> [128, (k, d)] bf16."""
    return np.ascontiguousarray(
        w.reshape(KH, 128, D).transpose(1, 0, 2).reshape(128, KH * D)
    )


def _route(xf: np.ndarray, moe_router: np.ndarray):
    """Top-2 routing on host. Returns per-expert (rows, weights)."""
    logits = xf @ moe_router                      # [T, E] f32
    top1 = np.argmax(logits, axis=1)
    tmp = logits.copy()
    tmp[np.arange(T), top1] = -np.inf
    top2 = np.argmax(tmp, axis=1)
    l1 = logits[np.arange(T), top1]
    l2 = logits[np.arange(T), top2]
    mx = np.maximum(l1, l2)
    e1 = np.exp(l1 - mx)
    e2 = np.exp(l2 - mx)
    s = e1 + e2
    w1 = (e1 / s).astype(np.float32)
    w2 = (e2 / s).astype(np.float32)
    per_expert = []
    for e in range(E):
        r1 = np.where(top1 == e)[0]
        r2 = np.where(top2 == e)[0]
        rows = np.concatenate([r1, r2])
        wts = np.concatenate([w1[r1], w2[r2]]).astype(np.float32)
        per_expert.append((rows, wts))
    return per_expert


def kernel(x, moe_router, moe_w13, moe_w2, _trace=False, _trace_kwargs=None):
    global last_results
    x = np.asarray(x)
    moe_router = np.asarray(moe_router)
    moe_w13 = np.asarray(moe_w13)
    moe_w2 = np.asarray(moe_w2)
    xf = np.ascontiguousarray(x.reshape(T, D).astype(np.float32))
    per_expert = _route(xf, np.asarray(moe_router, dtype=np.float32))

    cmax = max(len(rows) for rows, _ in per_expert)
    C = cmax + (cmax & 1)       # even, else exact (padding is pure overhead)
    C = max(C, 2 * TB)          # keep the block-0 / rest split well-formed

    nc = _NC_CACHE.get(C)
    if nc is None:
        nc = _build(C)
        _NC_CACHE[C] = nc

    xf_bf = xf.astype(ml_dtypes.bfloat16)
    in_maps = []
    for e in range(E):
        rows, _ = per_expert[e]
        xg = np.zeros((C, D), dtype=ml_dtypes.bfloat16)
        xg[: len(rows)] = xf_bf[rows]
        in_maps.append(
            {
                "xt": _stage_x(xg),
                "w13": _stage_w13(
                    np.asarray(moe_w13[e]).astype(ml_dtypes.bfloat16)
                ),
                "w2": _stage_w2(np.asarray(moe_w2[e]).astype(ml_dtypes.bfloat16)),
            }
        )

    res = run_bass_kernel_spmd(
        nc,
        in_maps,
        core_ids=list(range(E)),
        trace=_trace,
        **(_trace_kwargs or {}),
    )
    last_results = res

    out = np.zeros((T, D), dtype=np.float32)
    for e in range(E):
        rows, wts = per_expert[e]
        ote = res.results[e]["ot"]                # [D, C] bf16 (or f32)
        out[rows] += ote[:, : len(rows)].T.astype(np.float32) * wts[:, None]
    return out.reshape(B, S, D)
